# revision 66
# baseline (speedup 1.0000x reference)
"""Self-contained Trainium2 Bass kernel for the 3-layer GIN GNN (8 NeuronCores).

kernel(**inputs) takes FULL unsharded inputs, returns FULL [256, 1] f32 output.

Design:
- Graph-aligned node sharding: 32 graphs/core, each padded to `maxg` node
  slots (multiple of 128; 256 typically) -> npad = 32*maxg slots/core.
  Static pooling boundaries at multiples of maxg.
- Node tables split into half-shard tables A (slots [0, npad/2) of every
  core) and B (rest), so int16 gather indices cover 8*npad/2 = 32768 rows
  and the A-half AllGather can fire mid-layer, overlapping the rest of the
  layer's compute.
- Layers 2/3 run in two passes: pass 1 aggregates A-half messages into
  SBUF-resident zA while the B-half AllGather is still in flight; pass 2
  adds B-half aggregation and runs the MLP.
- Edges bucketed by owner core of dst per 128-node tile, split A/B by src
  slot half; chunk counts per (tile, half) maxed across cores and baked
  into one SPMD program.
- Messages gathered via nc.gpsimd.dma_gather from fp16 tables (1KB rows).
  Scatter-add = selection-matrix matmuls on the PE.
- MLPs in transposed orientation [feat(P), nodes(F)]; BN folded into
  per-partition scale/bias of ScalarE activations.
- No pooling collective: each core projects its own 32 graphs to a
  [32, 1] output; the host stitches the 8 outputs.
"""

import sys

sys.path.insert(0, "/opt/trn_rl_repo")

import numpy as np

import concourse.bass as bass  # noqa: F401
import concourse.mybir as mybir
import concourse.tile as tile
from concourse import bacc, library_config
from concourse.bass_utils import run_bass_kernel_spmd

NCORES = 8
G = 256
F_IN = 79
D = 400
BN_EPS = 1e-5

GPC = G // NCORES        # 32 graphs per core
MAXG_FLOOR = 256         # min padded nodes per graph (tests may lower)
P = 128
ELEM_H = 512             # fp16 elems per h row (1KB)
ELEM_X = 128             # fp16 elems per x row (256B)
OSL = 4                  # feature slices
SL = 100                 # slice width

# tunables for perf experiments
CFG = dict(msg_bufs=2, sl_bufs=4, zt_bufs=5, yt_bufs=4, ht_bufs=4, ot_bufs=4,
           nm_bufs=3, wk_bufs=3, swdge_queues=1, gather_rot=False)

F16 = mybir.dt.float16
F32 = mybir.dt.float32
F8 = mybir.dt.float8e3
I16 = mybir.dt.int16


# =================================================================== host prep
def _prep(inputs):
    x = np.asarray(inputs["x"], np.float32)
    edge_index = np.asarray(inputs["edge_index"]).astype(np.int64)
    batch = np.asarray(inputs["batch_index"]).astype(np.int64)
    n = x.shape[0]
    assert int(inputs["num_graphs"]) == G

    cnt = np.bincount(batch, minlength=G).astype(np.int64)
    gstart = np.zeros(G + 1, np.int64)
    np.cumsum(cnt, out=gstart[1:])

    maxg = max(MAXG_FLOOR, int(np.ceil(cnt.max() / P)) * P)
    win = 512 if 512 % maxg == 0 else maxg
    tpw = win // P                      # tiles per window
    gpw = win // maxg                   # graphs per window
    npad = GPC * maxg
    nrows = NCORES * npad
    nh = npad // 2                      # per-core rows per half-table
    assert npad % win == 0
    nwin = npad // win
    assert nwin % 2 == 0 and (nwin // 2) * win == nh
    ntile = npad // P

    g_of = batch
    rank = np.arange(n, dtype=np.int64) - gstart[g_of]
    core_of = g_of // GPC
    slot = (g_of % GPC) * maxg + rank
    row_of = core_of * npad + slot

    src = edge_index[0]
    dst = edge_index[1]
    e_core = core_of[dst]
    e_tile = slot[dst] // P
    e_dloc = slot[dst] % P
    # src addressed in quarter-tables: Q q = slots [q*nq, (q+1)*nq) of every
    # core; four independent AllGathers per layer boundary can then fire as
    # soon as each quarter's windows complete.
    nq = npad // 4
    assert nwin % 4 == 0 and (nwin // 4) * win == nq
    e_q = slot[src] // nq
    e_sidx = core_of[src] * nq + (slot[src] % nq)   # < 8*nq = 16384, int16-safe

    key = e_core * (ntile * 4) + e_tile * 4 + e_q
    order = np.argsort(key, kind="stable")
    skey = key[order]
    sidx = e_sidx[order]
    sdl = e_dloc[order]

    counts = np.bincount(key, minlength=NCORES * ntile * 4).reshape(
        NCORES, ntile, 4)
    cpt = np.maximum(np.ceil(counts / P).astype(np.int64).max(axis=0), 1)
    cpt_q = [[int(v) for v in cpt[:, q]] for q in range(4)]

    # chunk base per (window, quarter, tile-in-window), matching device layout
    ch_base = np.zeros((nwin, 4, tpw), np.int64)
    off = 0
    for w in range(nwin):
        for q in range(4):
            for t in range(tpw):
                ch_base[w, q, t] = off
                off += cpt_q[q][w * tpw + t]
    totch = off

    idx_all = np.zeros((NCORES, totch * P), np.int16)
    dst_all = np.full((NCORES, totch * P), -1.0, np.float32)

    bstart = np.searchsorted(skey, np.arange(NCORES * ntile * 4))
    bend = np.append(bstart[1:], len(skey))
    bstart = bstart.reshape(NCORES, ntile, 4)
    bend = bend.reshape(NCORES, ntile, 4)

    for c in range(NCORES):
        for w in range(nwin):
            for q in range(4):
                for t in range(tpw):
                    gt = w * tpw + t
                    b0, b1 = bstart[c, gt, q], bend[c, gt, q]
                    ne = b1 - b0
                    base = int(ch_base[w, q, t]) * P
                    idx_all[c, base:base + ne] = sidx[b0:b1].astype(np.int16)
                    dst_all[c, base:base + ne] = sdl[b0:b1].astype(np.float32)

    S_tot = totch * 8
    iw = idx_all.reshape(NCORES, totch * 8, 16).transpose(0, 2, 1)
    idx_wrapped = np.tile(iw, (1, 8, 1))                        # [C, 128, S]
    dw = dst_all.reshape(NCORES, totch, P).transpose(0, 2, 1)   # [C, 128, totch]

    x_nm = np.zeros((nrows, ELEM_X), np.float16)
    x_nm[row_of, :F_IN] = x.astype(np.float16)
    x_q = [np.ascontiguousarray(
        x_nm.reshape(NCORES, npad, ELEM_X)[:, q * nq:(q + 1) * nq]
        .reshape(-1, ELEM_X)) for q in range(4)]
    xT_g = x_nm[:, :P].T                                        # [128, nrows]

    real = np.zeros((NCORES, npad), np.float32)
    real[core_of, slot] = 1.0
    maskneg = (1.0 - real) * -60000.0

    w = {k: np.asarray(v, np.float32) for k, v in inputs.items()
         if k not in ("x", "edge_index", "batch_index", "num_graphs")}
    s1 = w["mlp1_bn_g"] / np.sqrt(w["mlp1_bn_v"] + BN_EPS)
    t1 = (w["mlp1_b1"] - w["mlp1_bn_m"]) * s1 + w["mlp1_bn_b"]
    s2 = w["mlp2_bn_g"] / np.sqrt(w["mlp2_bn_v"] + BN_EPS)
    t2 = (w["mlp2_b1"] - w["mlp2_bn_m"]) * s2 + w["mlp2_bn_b"]

    w1p = np.zeros((80, D), np.float16)
    w1p[:F_IN] = w["mlp1_w1"].astype(np.float16)

    def ksl(mat):       # [400, 400] -> [100(ki), 4(ko), 400(out)]
        return np.ascontiguousarray(
            mat.astype(np.float16).reshape(4, SL, D).transpose(1, 0, 2))

    def sb4(vec):       # [400] -> [100, 4]
        return np.ascontiguousarray(vec.astype(np.float32).reshape(4, SL).T)

    meta = dict(
        maxg=maxg, win=win, tpw=tpw, gpw=gpw, npad=npad, nrows=nrows,
        nh=nh, nq=nq, nwin=nwin, ntile=ntile,
        cpt_q=cpt_q,
        totch=totch, S_tot=S_tot,
        eps1=float(1.0 + np.asarray(inputs["eps1"], np.float32)[0]),
        eps2=float(1.0 + np.asarray(inputs["eps2"], np.float32)[0]),
        eps3=float(1.0 + np.asarray(inputs["eps3"], np.float32)[0]),
        out_b=float(w["out_b"][0]),
    )

    shared = {
        "x_q0": x_q[0], "x_q1": x_q[1], "x_q2": x_q[2], "x_q3": x_q[3],
        "w1p": w1p,
        "m1s": sb4(s1), "m1t": sb4(t1),
        "m1w2": ksl(w["mlp1_w2"]), "m1b2": sb4(w["mlp1_b2"]),
        "m2w1": ksl(w["mlp2_w1"]),
        "m2s": sb4(s2), "m2t": sb4(t2),
        "m2w2": ksl(w["mlp2_w2"]), "m2b2": sb4(w["mlp2_b2"]),
        "ow1": ksl(w["out1_w"]), "ob1": sb4(w["out1_b"]),
        "ow2": ksl(w["out2_w"]), "ob2": sb4(w["out2_b"]),
        "ow3": ksl(w["out3_w"]), "ob3": sb4(w["out3_b"]),
        "pwmax": np.ascontiguousarray(np.broadcast_to(
            w["out_w"][:D, 0].astype(np.float32)[None, :], (GPC, D))),
        "pwmean": np.ascontiguousarray(np.broadcast_to(
            w["out_w"][D:, 0].astype(np.float32)[None, :], (GPC, D))),
    }
    invcnt = (1.0 / np.maximum(cnt, 1)).astype(np.float32)[:, None]
    in_maps = []
    for c in range(NCORES):
        m = dict(shared)
        m["xT"] = np.ascontiguousarray(xT_g[:, c * npad:(c + 1) * npad])
        m["idxs"] = np.ascontiguousarray(idx_wrapped[c])
        m["dstf"] = np.ascontiguousarray(dw[c])
        m["invcnt"] = np.ascontiguousarray(invcnt[c * GPC:(c + 1) * GPC])
        m["maskneg"] = np.ascontiguousarray(np.broadcast_to(
            maskneg[c][None, :], (SL, npad))).astype(np.float16)
        m["maskmul"] = np.ascontiguousarray(np.broadcast_to(
            real[c][None, :], (SL, npad))).astype(np.float16)
        in_maps.append(m)
    return meta, in_maps


# =================================================================== device IR
def _build(meta, skip_coll=False, only_layer=None, pool_mode="full", reps=1):
    import contextlib

    nwin, nrows, nq = meta["nwin"], meta["nrows"], meta["nq"]
    npad, win, tpw, gpw = meta["npad"], meta["win"], meta["tpw"], meta["gpw"]
    maxg, ntile = meta["maxg"], meta["ntile"]
    cpt_q = meta["cpt_q"]
    S_tot, totch = meta["S_tot"], meta["totch"]
    # max chunks per (pair-of-tiles, quarter) -> msg buffer width
    Cmax = [max(sum(cpt_q[q][i:i + 2]) for i in range(0, len(cpt_q[q]), 2))
            for q in range(4)]

    nc = bacc.Bacc("TRN2", target_bir_lowering=False, debug=False,
                   num_devices=NCORES, num_swdge_queues=CFG["swdge_queues"])

    din = {}
    for name, shape, dt in [
        ("x_q0", [NCORES * nq, ELEM_X], F16),
        ("x_q1", [NCORES * nq, ELEM_X], F16),
        ("x_q2", [NCORES * nq, ELEM_X], F16),
        ("x_q3", [NCORES * nq, ELEM_X], F16), ("xT", [P, npad], F16),
        ("idxs", [P, S_tot], I16), ("dstf", [P, totch], F32),
        ("w1p", [80, D], F16), ("m1s", [SL, 4], F32), ("m1t", [SL, 4], F32),
        ("m1w2", [SL, 4, D], F16), ("m1b2", [SL, 4], F32),
        ("m2w1", [SL, 4, D], F16), ("m2s", [SL, 4], F32), ("m2t", [SL, 4], F32),
        ("m2w2", [SL, 4, D], F16), ("m2b2", [SL, 4], F32),
        ("ow1", [SL, 4, D], F16), ("ob1", [SL, 4], F32),
        ("ow2", [SL, 4, D], F16), ("ob2", [SL, 4], F32),
        ("ow3", [SL, 4, D], F16), ("ob3", [SL, 4], F32),
        ("pwmax", [GPC, D], F32), ("pwmean", [GPC, D], F32),
        ("invcnt", [GPC, 1], F32),
        ("maskneg", [SL, npad], F16), ("maskmul", [SL, npad], F16),
    ]:
        din[name] = nc.dram_tensor(name, shape, dt, kind="ExternalInput")
    out_t = nc.dram_tensor("out", [GPC, 1], F32, kind="ExternalOutput")

    eq = mybir.AluOpType.is_equal
    AF = mybir.ActivationFunctionType

    # chunk bases, same layout as host: ch_base[w][q][t]
    ch_base = []
    off = 0
    for w in range(nwin):
        qb = []
        for q in range(4):
            tb = []
            for t in range(tpw):
                tb.append(off)
                off += cpt_q[q][w * tpw + t]
            qb.append(tb)
        ch_base.append(qb)

    with tile.TileContext(nc) as tc:
        nc.gpsimd.load_library(library_config.mlp)
        with contextlib.ExitStack() as ctx:
            cst = ctx.enter_context(tc.tile_pool(name="cst", bufs=1))
            dram = ctx.enter_context(tc.tile_pool(name="drm", bufs=1, space="DRAM"))
            p_lo = ctx.enter_context(tc.tile_pool(name="p_lo", bufs=CFG["msg_bufs"]))
            p_hi = ctx.enter_context(tc.tile_pool(name="p_hi", bufs=CFG["msg_bufs"]))
            p_wk = ctx.enter_context(tc.tile_pool(name="p_wk", bufs=CFG["wk_bufs"]))
            p_sl = ctx.enter_context(tc.tile_pool(name="p_sl", bufs=CFG["sl_bufs"]))
            p_zt = ctx.enter_context(tc.tile_pool(name="p_zt", bufs=CFG["zt_bufs"]))
            p_yt = ctx.enter_context(tc.tile_pool(name="p_yt", bufs=CFG["yt_bufs"]))
            p_ht = ctx.enter_context(tc.tile_pool(name="p_ht", bufs=CFG["ht_bufs"]))
            p_ot = ctx.enter_context(tc.tile_pool(name="p_ot", bufs=CFG["ot_bufs"]))
            p_nm = ctx.enter_context(tc.tile_pool(name="p_nm", bufs=CFG["nm_bufs"]))
            p_pl = ctx.enter_context(tc.tile_pool(name="p_pl", bufs=2))
            p_fin = ctx.enter_context(tc.tile_pool(name="p_fin", bufs=1))
            ps_agg = ctx.enter_context(tc.tile_pool(name="ps_agg", bufs=2, space="PSUM"))
            ps_tr = ctx.enter_context(tc.tile_pool(name="ps_tr", bufs=2, space="PSUM"))
            ps_mm = ctx.enter_context(tc.tile_pool(name="ps_mm", bufs=4, space="PSUM"))

            # resident constants
            sb = {}
            for name in din:
                if name in ("x_q0", "x_q1", "x_q2", "x_q3", "xT", "invcnt",
                            "maskneg", "maskmul"):
                    continue
                t = cst.tile(list(din[name].shape), din[name].dtype,
                             name=f"sb_{name}")
                full = tuple(slice(None) for _ in din[name].shape)
                nc.sync.dma_start(t[full], din[name][full])
                sb[name] = t

            iota_r = cst.tile([P, P], F16, name="iota_r")
            nc.gpsimd.iota(iota_r[:], pattern=[[1, P]], base=0,
                           channel_multiplier=0,
                           allow_small_or_imprecise_dtypes=True)
            pcol = cst.tile([P, 1], F32, name="pcol")
            nc.gpsimd.iota(pcol[:], pattern=[[1, 1]], base=0,
                           channel_multiplier=1,
                           allow_small_or_imprecise_dtypes=True)
            ident = cst.tile([P, P], F16, name="ident")
            nc.vector.tensor_scalar(ident[:], iota_r[:], pcol[:, :1], None, eq)
            identf = cst.tile([P, P], F32, name="identf")
            nc.vector.tensor_copy(identf[:], ident[:])

            acc_max = [cst.tile([SL, GPC], F32, name=f"accm{o}") for o in range(OSL)]
            acc_sum = [cst.tile([SL, GPC], F32, name=f"accs{o}") for o in range(OSL)]

            def sel_tile(slot, dt=F16):
                s = p_sl.tile([P, P], dt, tag="sel", name="sel")
                nc.vector.tensor_scalar(
                    s[:], iota_r[:], sb["dstf"][:, slot:slot + 1], None, eq)
                return s

            def mm4(zts, wname, kp, ksl_n, act_pool, func, scale4, bias4, dt=F16):
                """For o in 0..3: act(sum_k W[k,o]^T @ zts[k]). Returns 4 tiles."""
                outs = []
                for o in range(OSL):
                    psy = ps_mm.tile([SL, win], F32, tag="psmm", name="psy")
                    for k in range(ksl_n):
                        lhsT = (sb[wname][:kp, k, o * SL:(o + 1) * SL]
                                if ksl_n > 1
                                else sb[wname][:kp, o * SL:(o + 1) * SL])
                        nc.tensor.matmul(psy[:, :], lhsT=lhsT,
                                         rhs=zts[k][:kp, :],
                                         start=(k == 0), stop=(k == ksl_n - 1))
                    t = act_pool.tile([SL, win], dt, tag=f"a_{act_pool.name}",
                                      name="actt")
                    sc = scale4[:, o:o + 1] if scale4 is not None else 1.0
                    nc.scalar.activation(t[:], psy[:, :], func,
                                         bias=bias4[:, o:o + 1], scale=sc)
                    outs.append(t)
                return outs

            # SBUF-resident A-half aggregation (+ eps*self), per dst tile
            zA_all = cst.tile([P, (npad // P) * D], F16, name="zA_all")

            def gather_q(w, q, table, elem, pool, tag):
                """Issue per-pair gathers for quarter q of window w.
                Returns ({pr: tile}, {pr: base_chunk})."""
                cpt = cpt_q[q]
                tiles, bases = {}, {}
                for pr in range((tpw + 1) // 2):
                    t0p, t1p = pr * 2, min(pr * 2 + 2, tpw)
                    b0 = ch_base[w][q][t0p]
                    ncc = sum(cpt[w * tpw + t0p: w * tpw + t1p])
                    m = pool.tile([P, Cmax[q], elem], F16, tag=tag, name=tag)
                    nc.gpsimd.dma_gather(
                        m[:, :ncc, :], table,
                        sb["idxs"][:, b0 * 8:(b0 + ncc) * 8],
                        ncc * P, ncc * P, elem, single_packet=False,
                        queue_num=0)
                    tiles[pr], bases[pr] = m, b0
                return tiles, bases

            def agg_q(w, t, q, tiles, bases, psum_ap, nfeat, first, last):
                """Accumulate quarter q's chunks of tile t into psum_ap."""
                gt = w * tpw + t
                ncc = cpt_q[q][gt]
                m, b0 = tiles[t // 2], bases[t // 2]
                for j in range(ncc):
                    slot = ch_base[w][q][t] + j
                    s = sel_tile(slot)
                    nc.tensor.matmul(
                        psum_ap, lhsT=s[:], rhs=m[:, slot - b0, :nfeat],
                        start=(first and j == 0), stop=(last and j == ncc - 1))

            def window_tail(layer, w, hot, shard_w, full):
                if layer < 3:
                    tpq = ntile // 4
                    for t in range(tpw):
                        hnm = p_nm.tile([P, D], F16, tag="hnm", name="hnm")
                        for fs in range(4):
                            ps2 = ps_tr.tile([P, SL], F16, tag="tr", name="ps2")
                            nc.tensor.transpose(
                                ps2[:, :], hot[fs][:, t * P:(t + 1) * P],
                                ident[:SL, :SL])
                            nc.any.tensor_copy(
                                out=hnm[:, fs * SL:(fs + 1) * SL], in_=ps2[:, :])
                        gt = w * tpw + t
                        r0 = (gt % tpq) * P
                        nc.sync.dma_start(
                            shard_w[gt // tpq][r0:r0 + P, :D], hnm[:])
                elif pool_mode != "none":
                    mneg = p_pl.tile([SL, win], F16, tag="mneg", name="mneg")
                    nc.sync.dma_start(mneg[:], din["maskneg"][:, w * win:(w + 1) * win])
                    mmul = p_pl.tile([SL, win], F16, tag="mmul", name="mmul")
                    nc.sync.dma_start(mmul[:], din["maskmul"][:, w * win:(w + 1) * win])
                    for o in range(OSL):
                        hm = p_pl.tile([SL, win], F32, tag="hm", name="hm")
                        nc.vector.tensor_tensor(
                            out=hm[:], in0=hot[o][:], in1=mneg[:],
                            op=mybir.AluOpType.add)
                        hs2 = p_pl.tile([SL, win], F32, tag="hs2", name="hs2")
                        nc.vector.tensor_tensor(
                            out=hs2[:], in0=hot[o][:], in1=mmul[:],
                            op=mybir.AluOpType.mult)
                        for gg in range(gpw):
                            gl = w * gpw + gg
                            nc.vector.tensor_reduce(
                                out=acc_max[o][:, gl:gl + 1],
                                in_=hm[:, gg * maxg:(gg + 1) * maxg],
                                axis=mybir.AxisListType.X,
                                op=mybir.AluOpType.max)
                            nc.vector.tensor_reduce(
                                out=acc_sum[o][:, gl:gl + 1],
                                in_=hs2[:, gg * maxg:(gg + 1) * maxg],
                                axis=mybir.AxisListType.X,
                                op=mybir.AluOpType.add)
                if layer < 3 and not skip_coll and (w + 1) % (nwin // 4) == 0:
                    # quarter-shard complete: queue its AllGather. The
                    # doorbell is EMITTED one window later (after that
                    # window's gathers are already issued) — the doorbell's
                    # input-ready wait sits on the gpsimd queue and would
                    # otherwise stall gather issuance at every quarter
                    # boundary until the producing windows drain.
                    return (w + 1) // (nwin // 4) - 1      # 0..3
                return None

            def fire_q(q, shard_w, full):
                # independent quarters progress concurrently on ncfw and
                # start moving bytes long before the layer ends
                nc.gpsimd.collective_compute(
                    "AllGather", mybir.AluOpType.bypass,
                    replica_groups=[list(range(NCORES))],
                    ins=[shard_w[q].opt()], outs=[full[q].opt()])

            for _rep in range(reps):
             h1_sw = [dram.tile([nq, ELEM_H], F16, name=f"h1_sw{q}")
                      for q in range(4)]
             h1_full = [dram.tile([NCORES * nq, ELEM_H], F16, name=f"h1_f{q}",
                                  addr_space="Shared") for q in range(4)]
             h2_sw = [dram.tile([nq, ELEM_H], F16, name=f"h2_sw{q}")
                      for q in range(4)]
             h2_full = [dram.tile([NCORES * nq, ELEM_H], F16, name=f"h2_f{q}",
                                  addr_space="Shared") for q in range(4)]
             carry = None
             for layer in ((1, 2, 3) if only_layer is None else only_layer):
                if layer == 1:
                    tabs = [din[f"x_q{q}"][:, :] for q in range(4)]
                    elem, eps = ELEM_X, meta["eps1"]
                elif layer == 2:
                    tabs = [t[:, :] for t in h1_full]
                    elem, eps = ELEM_H, meta["eps2"]
                else:
                    tabs = [t[:, :] for t in h2_full]
                    elem, eps = ELEM_H, meta["eps3"]
                shard_r = (None if layer == 1
                           else (h1_sw if layer == 2 else h2_sw))
                shard_w = (h1_sw if layer == 1
                           else (h2_sw if layer == 2 else None))
                full = (h1_full if layer == 1
                        else (h2_full if layer == 2 else None))
                tpq = ntile // 4

                if layer == 1:
                    # single-pass: x table is an input, no collective to wait on
                    pend = None
                    for w in range(nwin):
                        tq, bq = {}, {}
                        for q in range(4):
                            tq[q], bq[q] = gather_q(w, q, tabs[q], elem,
                                                    p_lo if q < 2 else p_hi,
                                                    f"g1q{q}")
                        if pend is not None:
                            fire_q(pend, shard_w, full)
                            pend = None
                        psz = ps_agg.tile([80, win], F32, tag="agg", name="psz")
                        for t in range(tpw):
                            gt = w * tpw + t
                            ncs = [cpt_q[q][gt] for q in range(4)]
                            tot = sum(ncs)
                            j = 0
                            for q in range(4):
                                for jq in range(ncs[q]):
                                    slot = ch_base[w][q][t] + jq
                                    rhs = tq[q][t // 2][:, slot - bq[q][t // 2], :80]
                                    s = sel_tile(slot)
                                    nc.tensor.matmul(
                                        psz[:, t * P:(t + 1) * P],
                                        lhsT=rhs, rhs=s[:],
                                        start=(j == 0), stop=(j == tot - 1))
                                    j += 1
                        xt = p_wk.tile([80, win], F16, tag="xt", name="xt")
                        nc.sync.dma_start(xt[:], din["xT"][:80, w * win:(w + 1) * win])
                        xs = p_wk.tile([80, win], F16, tag="xs", name="xs")
                        nc.scalar.mul(xs[:], xt[:], eps)
                        z1 = p_zt.tile([80, win], F16, tag="zt1", name="z1")
                        nc.vector.tensor_add(out=z1[:], in0=xs[:], in1=psz[:, :])
                        yt = mm4([z1], "w1p", 80, 1, p_yt, AF.Relu,
                                 sb["m1s"], sb["m1t"])
                        ht = mm4(yt, "m1w2", SL, 4, p_ht, AF.Relu, None,
                                 sb["m1b2"])
                        hot = mm4(ht, "ow1", SL, 4, p_ot, AF.Tanh, None,
                                  sb["ob1"])
                        r = window_tail(layer, w, hot, shard_w, full)
                        if r is not None:
                            pend = r
                    if pend is not None:
                        # defer the tail doorbell into the next layer's
                        # pass 1 (which only reads q0/q1 tables) so its
                        # input-ready wait doesn't stall gather issuance
                        carry = (pend, shard_w, full)
                else:
                    # pass 1: quarters 0+1 aggregation + eps*self -> zA_all.
                    # Overlaps the previous layer's tail AllGathers (q2, q3).
                    for w in range(nwin):
                        t0, b0_ = gather_q(w, 0, tabs[0], elem, p_lo, "g2q0")
                        t1, b1_ = gather_q(w, 1, tabs[1], elem, p_lo, "g2q1")
                        if carry is not None:
                            fire_q(*carry)
                            carry = None
                        for t in range(tpw):
                            gt = w * tpw + t
                            psa = ps_agg.tile([P, D], F32, tag="agg", name="psa")
                            agg_q(w, t, 0, t0, b0_, psa[:, :], D, True, False)
                            agg_q(w, t, 1, t1, b1_, psa[:, :], D, False, True)
                            hown = p_wk.tile([P, D], F16, tag="hown", name="hown")
                            r0 = (gt % tpq) * P
                            nc.sync.dma_start(
                                hown[:], shard_r[gt // tpq][r0:r0 + P, :D])
                            hsc = p_wk.tile([P, D], F16, tag="hsc", name="hsc")
                            nc.scalar.mul(hsc[:], hown[:], eps)
                            nc.vector.tensor_add(
                                out=zA_all[:, gt * D:(gt + 1) * D],
                                in0=hsc[:], in1=psa[:, :])
                    # pass 2: quarters 2+3 aggregation + MLP
                    pend = None
                    for w in range(nwin):
                        t2, b2_ = gather_q(w, 2, tabs[2], elem, p_hi, "g2q2")
                        t3, b3_ = gather_q(w, 3, tabs[3], elem, p_hi, "g2q3")
                        if pend is not None:
                            fire_q(pend, shard_w, full)
                            pend = None
                        ztiles = [p_zt.tile([SL, win], F16, tag="zt2",
                                            name=f"zt{k}") for k in range(4)]
                        for t in range(tpw):
                            gt = w * tpw + t
                            psa = ps_agg.tile([P, D], F32, tag="agg", name="psa")
                            agg_q(w, t, 2, t2, b2_, psa[:, :], D, True, False)
                            agg_q(w, t, 3, t3, b3_, psa[:, :], D, False, True)
                            znm = p_nm.tile([P, D], F16, tag="znm", name="znm")
                            nc.vector.tensor_add(
                                out=znm[:], in0=zA_all[:, gt * D:(gt + 1) * D],
                                in1=psa[:, :])
                            for fs in range(4):
                                pst = ps_tr.tile([SL, P], F16, tag="tr", name="pst")
                                nc.tensor.transpose(
                                    pst[:, :], znm[:, fs * SL:(fs + 1) * SL], ident[:])
                                nc.any.tensor_copy(
                                    out=ztiles[fs][:, t * P:(t + 1) * P], in_=pst[:, :])
                        yt = mm4(ztiles, "m2w1", SL, 4, p_yt, AF.Relu,
                                 sb["m2s"], sb["m2t"])
                        ht = mm4(yt, "m2w2", SL, 4, p_ht, AF.Relu, None,
                                 sb["m2b2"])
                        own, obn = ("ow2", "ob2") if layer == 2 else ("ow3", "ob3")
                        hot = mm4(ht, own, SL, 4, p_ot, AF.Tanh, None, sb[obn],
                                  dt=(F32 if layer == 3 else F16))
                        r = window_tail(layer, w, hot, shard_w, full)
                        if r is not None:
                            pend = r
                    if pend is not None:
                        carry = (pend, shard_w, full)

             if carry is not None:
                fire_q(*carry)
                carry = None
             # pooling finalize: local projection of this core's 32 graphs
             do_pool = (only_layer is None or 3 in only_layer) and pool_mode == "full"
             if do_pool:
                asm_mx = p_fin.tile([GPC, D], F32, tag="asm_mx", name="asm_mx")
                asm_sm = p_fin.tile([GPC, D], F32, tag="asm_sm", name="asm_sm")
                for o in range(OSL):
                    ps3 = ps_tr.tile([GPC, SL], F32, tag="tr", name="ps3")
                    nc.tensor.transpose(ps3[:, :], acc_max[o][:, :], identf[:SL, :SL])
                    nc.any.tensor_copy(out=asm_mx[:, o * SL:(o + 1) * SL],
                                       in_=ps3[:, :])
                    ps4 = ps_tr.tile([GPC, SL], F32, tag="tr", name="ps4")
                    nc.tensor.transpose(ps4[:, :], acc_sum[o][:, :], identf[:SL, :SL])
                    nc.any.tensor_copy(out=asm_sm[:, o * SL:(o + 1) * SL],
                                       in_=ps4[:, :])
                ic = p_fin.tile([GPC, 1], F32, tag="ic", name="ic")
                nc.sync.dma_start(ic[:], din["invcnt"][:, :])
                t1 = p_fin.tile([GPC, D], F32, tag="t1", name="t1")
                nc.vector.tensor_tensor(
                    out=t1[:], in0=asm_mx[:, :], in1=sb["pwmax"][:, :],
                    op=mybir.AluOpType.mult)
                d1 = p_fin.tile([GPC, 1], F32, tag="d1", name="d1")
                nc.vector.tensor_reduce(out=d1[:], in_=t1[:],
                                        axis=mybir.AxisListType.X,
                                        op=mybir.AluOpType.add)
                t2 = p_fin.tile([GPC, D], F32, tag="t2", name="t2")
                nc.vector.tensor_tensor(
                    out=t2[:], in0=asm_sm[:, :], in1=sb["pwmean"][:, :],
                    op=mybir.AluOpType.mult)
                d2 = p_fin.tile([GPC, 1], F32, tag="d2", name="d2")
                nc.vector.tensor_reduce(out=d2[:], in_=t2[:],
                                        axis=mybir.AxisListType.X,
                                        op=mybir.AluOpType.add)
                nc.vector.tensor_tensor(out=d2[:], in0=d2[:], in1=ic[:],
                                        op=mybir.AluOpType.mult)
                nc.vector.tensor_add(out=d1[:], in0=d1[:], in1=d2[:])
                nc.vector.tensor_scalar_add(d1[:], d1[:], meta["out_b"])
                nc.sync.dma_start(out_t[:, :], d1[:])

    nc.finalize()
    return nc


_CACHE = {}


def build_all(inputs):
    """Returns (nc, meta, in_maps); caches the compiled program."""
    meta, in_maps = _prep(inputs)
    key = (meta["nwin"], meta["totch"],
           tuple(tuple(c) for c in meta["cpt_q"]),
           meta["eps1"], meta["eps2"], meta["eps3"], meta["out_b"])
    if key not in _CACHE:
        _CACHE.clear()
        _CACHE[key] = _build(meta)
    return _CACHE[key], meta, in_maps


def kernel(**inputs):
    nc, meta, in_maps = build_all(inputs)
    res = run_bass_kernel_spmd(nc, in_maps, core_ids=list(range(NCORES)))
    return np.concatenate(
        [np.asarray(res.results[c]["out"], np.float32) for c in range(NCORES)],
        axis=0)

